# revision 17
# baseline (speedup 1.0000x reference)
"""Trainium2 Bass kernel for cross-attention (cosine-normalized, 8 heads).

Reference computation (full inputs x,y [1,4096,64]):
  q = x@Wq+bq ; k,v = split(y@Wkv+bkv) ; per head (8 heads, dim 8):
  attn = softmax(l2norm(q) @ l2norm(k)^T) ; out = attn@v
  result = concat_heads(out) @ We + be

Sharding: one head per NeuronCore (8 heads / 8 cores), SPMD program with
per-core weight slices. Each core returns resT_h = (out_h @ We_h + be/8)^T
as [64, 4096]; the host sums over cores and transposes.

v2 restructure (baseline was 217.8us; exp stream itself is ~132us of ACT
work and near-irreducible, so this version attacks the 52.5us prologue
and 23us epilogue that were serialized around it):
  - fp16 activations/weights end-to-end: halves input DMA bytes, enables
    2x DVE perf modes, ~8x better elementwise precision than bf16.
  - Inputs DMA'd in quarters on BOTH HWDGE queues (x on scalar, y on
    sync) so per-quarter projection pipelines start as data lands; all
    weights arrive in one packed [128,152] DMA.
  - Prologue engine balance: ACT does q/k psum copies + v copies (ScalarE
    is fastest at PSUM reads) then rsqrts; DVE does the f16 squares +
    one k copy + normalize muls. Exp table is warmed right after the
    rsqrts so the first EXP pays no table load.
  - Norms: selector matmuls (K=8) pack per-block sums into partitions,
    one Rsqrt per side, one stride-0 broadcast DMA per side, f16 2x
    normalize muls (block 0 first to unblock the main loop).
  - Epilogue pipelined INTO the main loop: when block j finishes
    accumulating, its reciprocal/broadcast/normalize/projection/store
    chain drips one stage per (q-block, group) entry under block j+1's
    exp stream. Only block 7's ~5us chain trails the last exp.
"""

import sys

import numpy as np

for _p in ("/opt/trn_rl_repo",):
    if _p not in sys.path:
        sys.path.insert(0, _p)

from contextlib import ExitStack

import concourse.bass as bass
import concourse.tile as tile
from concourse import bacc, mybir
from concourse.bass import ts
from concourse.bass_utils import run_bass_kernel_spmd

F32 = mybir.dt.float32
F16 = mybir.dt.float16
AF = mybir.ActivationFunctionType

HW = 4096          # sequence length
C = 64             # model dim
H = 8              # heads
D = 8              # head dim
QB = 512           # q block
NQB = HW // QB     # 8
KC = 128           # k chunk
NKC = HW // KC     # 32
GROUPS = [3] * 10 + [2]   # k-chunks per exp/ACT group (32 total)
GMAX = max(GROUPS)
VW = D + 1         # v + ones column
QTR = HW // 4      # DMA quarter

# wpack column layout
WQ0, WK0, WV0, WE0, SEL0, SELR0 = 0, 8, 16, 24, 88, 152
WPC = 216

_BUILT = None
TRACE = False
LAST_RESULTS = None


def _body(ctx, tc, dram):
    nc = tc.nc
    xTe_d, yTe_d, wpack_d, oneden_d, out_d = dram
    U16 = mybir.dt.uint16

    const = ctx.enter_context(tc.tile_pool(name="const", bufs=1))
    expp = ctx.enter_context(tc.tile_pool(name="exps", bufs=4))
    ps_s = ctx.enter_context(tc.tile_pool(name="ps_s", bufs=2, space="PSUM"))
    ps_o = ctx.enter_context(tc.tile_pool(name="ps_o", bufs=2, space="PSUM"))

    xTe = const.tile([KC, HW], F16)    # host: x^T rows 0-63, ones row 64
    yTe = const.tile([KC, HW], F16)
    wpack = const.tile([KC, WPC], F16)
    qT = const.tile([D, HW], F16)
    kT = const.tile([D, HW], F16)
    sqq8 = const.tile([D, HW], F16)    # q squares
    sqk = const.tile([D, HW], F16)
    prj = const.tile([KC, HW], F16)    # normalized-out staging for proj
    qTn = const.tile([KC, HW], F16)
    kTn = const.tile([KC, HW], F16)
    vext = const.tile([KC, VW * NKC], F16)
    inv_q = const.tile([D, QB], F16)
    inv_k = const.tile([D, QB], F16)
    rep_q = const.tile([D, HW], F16)
    rep_k = xTe[0:D, :]                # reuse: x rows dead after q proj
    oTe = const.tile([VW, HW], F32)
    oneden_t = const.tile([VW, D], F32)
    deno = const.tile([1, QB], F32)    # per-block denom row at partition 0
    rcpo = const.tile([1, QB], F32)    # its reciprocal
    invr = const.tile([VW, QB], F32)   # prologue sqrt scratch
    repE = const.tile([D, HW], F32)    # epilogue inv-den broadcast
    resT = const.tile([C, HW], F32)

    # zero padded rows once (gpsimd; overlaps startup + DMA-in). kTn rows
    # >=D must be true zeros (stationary in score matmuls); qTn/sqq pads
    # must at least be finite.
    nc.gpsimd.memset(vext[:], 1.0)           # ones col -> softmax denom
    nc.gpsimd.memset(kTn[:].bitcast(U16), 0)
    nc.gpsimd.memset(qTn[:].bitcast(U16), 0)
    nc.gpsimd.memset(prj[:].bitcast(U16), 0)

    # preload the sqrt activation table during the DMA phase
    warm = const.tile([1, 1], F32)
    nc.vector.memset(warm[:], 1.0)
    nc.scalar.sqrt(warm[:], warm[:])

    # PE p-state warmup: ~8 junk matmuls keep TensorE busy through the
    # DMA phase so the clock is ramped when the projections arrive
    wps = ps_o.tile([KC, QB], F32, tag="pso", name="warmps")
    for _ in range(8):
        nc.tensor.matmul(wps[0:D, 0:VW * NKC], vext[:, 0:D], vext[:],
                         start=True, stop=True)

    # ---------------- input DMAs: x on scalar queue, y on sync ----------
    nc.sync.dma_start(wpack[:], wpack_d)
    nc.sync.dma_start(oneden_t[:], oneden_d)
    for qq in range(4):
        nc.scalar.dma_start(xTe[:, ts(qq, QTR)], xTe_d[:, ts(qq, QTR)])
        nc.sync.dma_start(yTe[:, ts(qq, QTR)], yTe_d[:, ts(qq, QTR)])

    wq = wpack[:, WQ0:WQ0 + D]
    wk = wpack[:, WK0:WK0 + D]
    wv = wpack[:, WV0:WV0 + D]
    webe = wpack[:, WE0:WE0 + C]

    # ------------- per-quarter projection pipelines (q, k, v) -----------
    # psum copies on ACT (ScalarE has the fastest PSUM path) except the
    # last k quarter on DVE; f16 squares on DVE at 2x.
    vex3 = vext[:].rearrange("p (c v) -> p c v", v=VW)
    for qq in range(4):
        sl = ts(qq, QTR)
        qp = ps_s.tile([KC, GMAX * QB], F32, tag="pss", name=f"qp{qq}")
        nc.tensor.matmul(qp[0:D, 0:QB], wq, xTe[:, ts(2 * qq, QB)],
                         start=True, stop=True)
        nc.tensor.matmul(qp[0:D, QB:QTR], wq, xTe[:, ts(2 * qq + 1, QB)],
                         start=True, stop=True)
        kp = ps_s.tile([KC, GMAX * QB], F32, tag="pss", name=f"kp{qq}")
        nc.tensor.matmul(kp[0:D, 0:QB], wk, yTe[:, ts(2 * qq, QB)],
                         start=True, stop=True)
        nc.tensor.matmul(kp[0:D, QB:QTR], wk, yTe[:, ts(2 * qq + 1, QB)],
                         start=True, stop=True)
        vp = ps_s.tile([KC, GMAX * QB], F32, tag="pss", name=f"vp{qq}")
        for u in range(8):
            c = 8 * qq + u
            nc.tensor.matmul(vp[:, ts(u, D)], yTe[:, ts(c, KC)], wv,
                             start=True, stop=True)
        # copies: ACT (q, k quarters 0-2, v), DVE (k quarter 3)
        nc.scalar.activation(qT[:, sl], qp[0:D, 0:QTR], AF.Copy)
        if qq < 3:
            nc.scalar.activation(kT[:, sl], kp[0:D, 0:QTR], AF.Copy)
        else:
            nc.vector.tensor_copy(kT[:, sl], kp[0:D, 0:QTR])
        vp3 = vp[:, 0:8 * D].rearrange("p (c v) -> p c v", v=D)
        nc.scalar.activation(vex3[:, 8 * qq:8 * qq + 8, 0:D], vp3, AF.Copy)
        # f16 squares (DVE 2x)
        nc.vector.tensor_mul(sqq8[:, sl], qT[:, sl], qT[:, sl])
        nc.vector.tensor_mul(sqk[:, sl], kT[:, sl], kT[:, sl])

    # ---------------- inverse norms ----------------
    # selector matmuls (K=8) pack per-block column sums into partitions
    ssq_k = ps_o.tile([D, QB], F32, tag="pso", name="ssq_k")
    ssq_q = ps_s.tile([KC, GMAX * QB], F32, tag="pss", name="ssq_q")
    for ssq, sq in ((ssq_k[:], sqk[:]), (ssq_q[0:D, 0:QB], sqq8[:])):
        for j in range(NQB):
            sel_j = wpack[0:D, SEL0 + D * j:SEL0 + D * (j + 1)]
            nc.tensor.matmul(ssq, sel_j, sq[:, ts(j, QB)],
                             start=(j == 0), stop=(j == NQB - 1))
    # inv = 1/sqrt(ssq): ACT sqrt -> DVE fast reciprocal (fp32) -> f16
    # (Rsqrt activation is blocked in bass for accuracy); scratches reuse
    # prologue-dead f32 tiles.
    nc.scalar.sqrt(invr[0:D, :], ssq_k[:])
    nc.scalar.sqrt(oTe[0:D, 0:QB], ssq_q[0:D, 0:QB])
    # exp table load: input aliases the sqrt output so the scheduler
    # cannot hoist it before the sqrts (which would thrash table sets)
    nc.scalar.activation(warm[:], oTe[0:1, 0:1], AF.Exp)
    nc.vector.reciprocal_approx_fast(repE[:, 0:QB], invr[0:D, :])
    nc.vector.tensor_copy(inv_k[:], repE[:, 0:QB])
    nc.vector.reciprocal_approx_fast(repE[:, QB:2 * QB], oTe[0:D, 0:QB])
    nc.vector.tensor_copy(inv_q[:], repE[:, QB:2 * QB])

    # prj ones row (for the be/8 bias fold) copied from xTe's host ones
    # row — engine memsets cannot start at partition 8
    nc.sync.dma_start(prj[D:D + 1, :], xTe[C:C + 1, :])

    # replicate inverse norms across partitions with PE row-broadcast
    # matmuls (selr one-hot rows) + per-block normalize muls: an
    # all-engine chain, no DMA completion latency. k block 0 and q
    # block 0 first to unblock the first score matmul.
    def nmul(side_inv, src, dst, j, nm):
        rb = ps_o.tile([D, QB], F32, tag="pso", name=nm)
        selr_j = wpack[0:D, SELR0 + D * j:SELR0 + D * (j + 1)]
        nc.tensor.matmul(rb[:], selr_j, side_inv[:], start=True, stop=True)
        nc.vector.tensor_mul(dst[0:D, ts(j, QB)], src[:, ts(j, QB)], rb[:])

    nmul(inv_k, kT, kTn, 0, "rbk0")
    nmul(inv_q, qT, qTn, 0, "rbq0")
    for j in range(1, NQB):
        nmul(inv_k, kT, kTn, j, f"rbk{j}")
    for j in range(1, NQB):
        nmul(inv_q, qT, qTn, j, f"rbq{j}")

    # ---------------- main attention loop + dripped epilogue ------------
    dscr_e, _ = tc.tile([NQB, QB], F32, space="DRAM", name="dscr_e")
    seq = []
    for j in range(NQB):
        c = 0
        for g in GROUPS:
            seq.append((j, c, g))
            c += g
    pos = [None] * NQB
    pss = [None] * len(seq)

    def mm1(i):
        j, c, g = seq[i]
        ps = ps_s.tile([KC, GMAX * QB], F32, tag="pss", name=f"pss{i}")
        pss[i] = ps
        for u in range(g):
            nc.tensor.matmul(ps[:, ts(u, QB)], kTn[:, ts(c + u, KC)],
                             qTn[:, ts(j, QB)], start=True, stop=True)

    def epi_stages(j):
        """Per-block epilogue: 1/den, broadcast, normalize, project,
        store — dripped one stage per subsequent loop entry."""
        projps = [None]

        def s_den(j=j):
            # engine ops need quadrant-aligned partition bases, so the
            # denom row (partition 8) moves to partition 0 by DMA first
            nc.sync.dma_start(deno[:], oTe[D:D + 1, ts(j, QB)])

        def s_recip(j=j):
            nc.vector.reciprocal_approx_fast(rcpo[:], deno[:])

        def s_wr(j=j):
            nc.sync.dma_start(dscr_e[j:j + 1, :], rcpo[:])

        def s_bcast(j=j):
            nc.sync.dma_start(repE[:, ts(j, QB)],
                              dscr_e[j:j + 1, :].to_broadcast((D, QB)))

        def s_mul(j=j):
            nc.vector.tensor_mul(prj[0:D, ts(j, QB)], oTe[0:D, ts(j, QB)],
                                 repE[:, ts(j, QB)])

        def s_proj(j=j):
            ps = ps_o.tile([C, QB], F32, tag="pso", name=f"proj{j}")
            nc.tensor.matmul(ps[:], webe, prj[:, ts(j, QB)], start=True,
                             stop=True)
            projps[0] = ps

        def s_copy(j=j):
            nc.vector.tensor_copy(resT[:, ts(j, QB)], projps[0][:])

        def s_out(j=j):
            nc.sync.dma_start(out_d[:, ts(j, QB)], resT[:, ts(j, QB)])

        def s_den7(j=j):
            # last block: PSUM is free, broadcast the denom row with one
            # fp32 matmul (oneden row-8 selector) instead of 3 chained
            # DMAs with ~1.3us completion latency each
            ps = ps_s.tile([D, QB], F32, tag="pss", name="den7")
            nc.tensor.matmul(ps[:], oneden_t[:], oTe[:, ts(j, QB)],
                             start=True, stop=True)
            projps[0] = ps

        def s_recip7(j=j):
            nc.vector.reciprocal_approx_fast(repE[:, ts(j, QB)],
                                             projps[0][:])

        if j == NQB - 1:
            return [s_den7, s_recip7, s_mul, s_proj, s_copy, s_out]
        return [s_den, s_recip, s_wr, s_bcast, s_mul, s_proj, s_copy, s_out]

    pend = []

    mm1(0)
    for i, (j, c, g) in enumerate(seq):
        if pos[j] is None:
            pos[j] = ps_o.tile([VW, QB], F32, tag="pso", name=f"po{j}")
        if i + 1 < len(seq):
            mm1(i + 1)
        ps = pss[i]
        es = expp.tile([KC, GMAX * QB], F16, tag="es")
        nc.scalar.activation(es[:, 0:g * QB], ps[:, 0:g * QB], AF.Exp)
        for u in range(g):
            cc = c + u
            nc.tensor.matmul(pos[j][:], vext[:, cc * VW:(cc + 1) * VW],
                             es[:, ts(u, QB)],
                             start=(cc == 0), stop=(cc == NKC - 1))
        pss[i] = None
        if c + g == NKC:
            nc.vector.tensor_copy(oTe[:, ts(j, QB)], pos[j][:])
            pend.append(epi_stages(j))
        elif pend:
            pend[0].pop(0)()
            if not pend[0]:
                pend.pop(0)
    while pend:
        pend[0].pop(0)()
        if not pend[0]:
            pend.pop(0)


def _build():
    global _BUILT
    if _BUILT is not None:
        return _BUILT
    nc = bacc.Bacc("TRN2", target_bir_lowering=False, debug=False, num_devices=H)
    xTe_d = nc.dram_tensor("xTe", [KC, HW], F16, kind="ExternalInput").ap()
    yTe_d = nc.dram_tensor("yTe", [KC, HW], F16, kind="ExternalInput").ap()
    wpack_d = nc.dram_tensor("wpack", [KC, WPC], F16, kind="ExternalInput").ap()
    oneden_d = nc.dram_tensor("oneden", [VW, D], F32,
                              kind="ExternalInput").ap()
    out_d = nc.dram_tensor("resT", [C, HW], F32, kind="ExternalOutput").ap()
    with tile.TileContext(nc) as tc, ExitStack() as ctx:
        _body(ctx, tc, (xTe_d, yTe_d, wpack_d, oneden_d, out_d[:]))
    nc.compile()
    _BUILT = nc
    return nc


def make_in_maps(x, y, Wq, bq, Wkv, bkv, We, be):
    x, y, Wq, bq, Wkv, bkv, We, be = (
        np.asarray(a, np.float32) for a in (x, y, Wq, bq, Wkv, bkv, We, be))
    ones = np.ones((1, HW), np.float32)
    zrows = np.zeros((KC - C - 1, HW), np.float32)
    xTe = np.vstack([x[0].T, ones, zrows]).astype(np.float16)
    yTe = np.vstack([y[0].T, ones, zrows]).astype(np.float16)
    sel = np.zeros((KC, C), np.float32)
    selr = np.zeros((KC, C), np.float32)
    for j in range(NQB):
        sel[0:D, D * j + j] = 1.0          # col j of group j: ones rows 0-7
        selr[j, D * j:D * (j + 1)] = 1.0   # row j of group j: all ones
    oneden = np.zeros((VW, D), np.float32)
    oneden[D, :] = 1.0
    in_maps = []
    for h in range(H):
        sl = slice(h * D, (h + 1) * D)
        slv = slice(C + h * D, C + (h + 1) * D)
        zc = np.zeros((KC - C - 1, D), np.float32)
        wqe = np.vstack([Wq[:, sl], bq[None, sl], zc])
        wke = np.vstack([Wkv[:, sl], bkv[None, sl], zc])
        wve = np.vstack([Wkv[:, slv], bkv[None, slv], zc])
        webe = np.vstack([We[sl, :], be[None, :] / H,
                          np.zeros((KC - VW, C), np.float32)])
        wpack = np.concatenate([wqe, wke, wve, webe, sel, selr], axis=1)
        in_maps.append({
            "xTe": xTe,
            "yTe": yTe,
            "wpack": np.ascontiguousarray(wpack.astype(np.float16)),
            "oneden": oneden,
        })
    return in_maps


def kernel(x, y, Wq, bq, Wkv, bkv, We, be):
    global LAST_RESULTS
    nc = _build()
    in_maps = make_in_maps(x, y, Wq, bq, Wkv, bkv, We, be)
    res = run_bass_kernel_spmd(nc, in_maps, core_ids=list(range(H)), trace=TRACE)
    LAST_RESULTS = res
    acc = np.zeros((C, HW), np.float64)
    for r in res.results:
        acc += r["resT"]
    return np.ascontiguousarray(acc.T[None]).astype(np.float32)


# revision 18
# speedup vs baseline: 1.0452x; 1.0452x over previous
"""Trainium2 Bass kernel for cross-attention (cosine-normalized, 8 heads).

Reference computation (full inputs x,y [1,4096,64]):
  q = x@Wq+bq ; k,v = split(y@Wkv+bkv) ; per head (8 heads, dim 8):
  attn = softmax(l2norm(q) @ l2norm(k)^T) ; out = attn@v
  result = concat_heads(out) @ We + be

Sharding: one head per NeuronCore (8 heads / 8 cores), SPMD program with
per-core weight slices. Each core returns resT_h = (out_h @ We_h + be/8)^T
as [64, 4096]; the host sums over cores and transposes.

v2 restructure (baseline was 217.8us; exp stream itself is ~132us of ACT
work and near-irreducible, so this version attacks the 52.5us prologue
and 23us epilogue that were serialized around it):
  - fp16 activations/weights end-to-end: halves input DMA bytes, enables
    2x DVE perf modes, ~8x better elementwise precision than bf16.
  - Inputs DMA'd in quarters on BOTH HWDGE queues (x on scalar, y on
    sync) so per-quarter projection pipelines start as data lands; all
    weights arrive in one packed [128,152] DMA.
  - Prologue engine balance: ACT does q/k psum copies + v copies (ScalarE
    is fastest at PSUM reads) then rsqrts; DVE does the f16 squares +
    one k copy + normalize muls. Exp table is warmed right after the
    rsqrts so the first EXP pays no table load.
  - Norms: selector matmuls (K=8) pack per-block sums into partitions,
    one Rsqrt per side, one stride-0 broadcast DMA per side, f16 2x
    normalize muls (block 0 first to unblock the main loop).
  - Epilogue pipelined INTO the main loop: when block j finishes
    accumulating, its reciprocal/broadcast/normalize/projection/store
    chain drips one stage per (q-block, group) entry under block j+1's
    exp stream. Only block 7's ~5us chain trails the last exp.
"""

import sys

import numpy as np

for _p in ("/opt/trn_rl_repo",):
    if _p not in sys.path:
        sys.path.insert(0, _p)

from contextlib import ExitStack

import concourse.bass as bass
import concourse.tile as tile
from concourse import bacc, mybir
from concourse.bass import ts
from concourse.bass_utils import run_bass_kernel_spmd

F32 = mybir.dt.float32
F16 = mybir.dt.float16
AF = mybir.ActivationFunctionType

HW = 4096          # sequence length
C = 64             # model dim
H = 8              # heads
D = 8              # head dim
QB = 512           # q block
NQB = HW // QB     # 8
KC = 128           # k chunk
NKC = HW // KC     # 32
GROUPS = [2] + [3] * 10   # small group first: at block joints the
                          # PE then always fits its 5 matmuls
GMAX = max(GROUPS)
VW = D + 1         # v + ones column
QTR = HW // 4      # DMA quarter

# wpack column layout
WQ0, WK0, WV0, WE0, SEL0, SELR0 = 0, 8, 16, 24, 88, 152
WPC = 216

_BUILT = None
TRACE = False
LAST_RESULTS = None


def _body(ctx, tc, dram):
    nc = tc.nc
    xTe_d, yTe_d, wpack_d, oneden_d, out_d = dram
    U16 = mybir.dt.uint16

    const = ctx.enter_context(tc.tile_pool(name="const", bufs=1))
    expp = ctx.enter_context(tc.tile_pool(name="exps", bufs=4))
    ps_s = ctx.enter_context(tc.tile_pool(name="ps_s", bufs=2, space="PSUM"))
    ps_o = ctx.enter_context(tc.tile_pool(name="ps_o", bufs=2, space="PSUM"))

    xTe = const.tile([KC, HW], F16)    # host: x^T rows 0-63, ones row 64
    yTe = const.tile([KC, HW], F16)
    wpack = const.tile([KC, WPC], F16)
    qT = const.tile([D, HW], F16)
    kT = const.tile([D, HW], F16)
    sqq8 = const.tile([D, HW], F16)    # q squares
    sqk = const.tile([D, HW], F16)
    prj = const.tile([KC, HW], F16)    # normalized-out staging for proj
    qTn = const.tile([KC, HW], F16)
    kTn = const.tile([KC, HW], F16)
    vext = const.tile([KC, VW * NKC], F16)
    inv_q = const.tile([D, QB], F16)
    inv_k = const.tile([D, QB], F16)
    rep_q = const.tile([D, HW], F16)
    rep_k = xTe[0:D, :]                # reuse: x rows dead after q proj
    oTe = const.tile([VW, HW], F32)
    oneden_t = const.tile([VW, D], F32)
    deno = const.tile([1, QB], F32)    # per-block denom row at partition 0
    rcpo = const.tile([1, QB], F32)    # its reciprocal
    invr = const.tile([VW, QB], F32)   # prologue sqrt scratch
    repE = const.tile([D, HW], F32)    # epilogue inv-den broadcast
    resT = const.tile([C, HW], F32)

    # zero padded rows once (gpsimd; overlaps startup + DMA-in). kTn rows
    # >=D must be true zeros (stationary in score matmuls); qTn/sqq pads
    # must at least be finite.
    nc.gpsimd.memset(vext[:], 1.0)           # ones col -> softmax denom
    nc.gpsimd.memset(kTn[:].bitcast(U16), 0)
    nc.gpsimd.memset(qTn[:].bitcast(U16), 0)
    nc.gpsimd.memset(prj[:].bitcast(U16), 0)

    # preload the sqrt activation table during the DMA phase
    warm = const.tile([1, 1], F32)
    nc.vector.memset(warm[:], 1.0)
    nc.scalar.sqrt(warm[:], warm[:])

    # PE p-state warmup: ~8 junk matmuls keep TensorE busy through the
    # DMA phase so the clock is ramped when the projections arrive
    wps = ps_o.tile([KC, QB], F32, tag="pso", name="warmps")
    for _ in range(8):
        nc.tensor.matmul(wps[0:D, 0:VW * NKC], vext[:, 0:D], vext[:],
                         start=True, stop=True)

    # ------------- input DMAs (x0/x1 scalar; rest on sync) --------------
    # only two DMAs ever sit on the scalar queue so ACT table loads and
    # copies never starve the x stream
    nc.scalar.dma_start(xTe[:, ts(0, QTR)], xTe_d[:, ts(0, QTR)])
    nc.scalar.dma_start(xTe[:, ts(1, QTR)], xTe_d[:, ts(1, QTR)])
    nc.sync.dma_start(wpack[:], wpack_d)
    nc.sync.dma_start(oneden_t[:], oneden_d)
    nc.sync.dma_start(yTe[:, ts(0, QTR)], yTe_d[:, ts(0, QTR)])
    nc.sync.dma_start(yTe[:, ts(1, QTR)], yTe_d[:, ts(1, QTR)])
    nc.sync.dma_start(xTe[:, ts(2, QTR)], xTe_d[:, ts(2, QTR)])
    nc.sync.dma_start(yTe[:, ts(2, QTR)], yTe_d[:, ts(2, QTR)])
    nc.sync.dma_start(xTe[:, ts(3, QTR)], xTe_d[:, ts(3, QTR)])
    nc.sync.dma_start(yTe[:, ts(3, QTR)], yTe_d[:, ts(3, QTR)])

    wq = wpack[:, WQ0:WQ0 + D]
    wk = wpack[:, WK0:WK0 + D]
    wv = wpack[:, WV0:WV0 + D]
    webe = wpack[:, WE0:WE0 + C]

    # ------------- per-quarter projection pipelines (q, k, v) -----------
    # psum copies on ACT (ScalarE has the fastest PSUM path) except the
    # last k quarter on DVE; f16 squares on DVE at 2x.
    vex3 = vext[:].rearrange("p (c v) -> p c v", v=VW)
    for qq in range(4):
        sl = ts(qq, QTR)
        qp = ps_s.tile([KC, GMAX * QB], F32, tag="pss", name=f"qp{qq}")
        nc.tensor.matmul(qp[0:D, 0:QB], wq, xTe[:, ts(2 * qq, QB)],
                         start=True, stop=True)
        nc.tensor.matmul(qp[0:D, QB:QTR], wq, xTe[:, ts(2 * qq + 1, QB)],
                         start=True, stop=True)
        kp = ps_s.tile([KC, GMAX * QB], F32, tag="pss", name=f"kp{qq}")
        nc.tensor.matmul(kp[0:D, 0:QB], wk, yTe[:, ts(2 * qq, QB)],
                         start=True, stop=True)
        nc.tensor.matmul(kp[0:D, QB:QTR], wk, yTe[:, ts(2 * qq + 1, QB)],
                         start=True, stop=True)
        vp = ps_s.tile([KC, GMAX * QB], F32, tag="pss", name=f"vp{qq}")
        for u in range(8):
            c = 8 * qq + u
            nc.tensor.matmul(vp[:, ts(u, D)], yTe[:, ts(c, KC)], wv,
                             start=True, stop=True)
        # copies: ACT (q, k quarters 0-2, v), DVE (k quarter 3)
        nc.scalar.activation(qT[:, sl], qp[0:D, 0:QTR], AF.Copy)
        if qq < 3:
            nc.scalar.activation(kT[:, sl], kp[0:D, 0:QTR], AF.Copy)
        else:
            nc.vector.tensor_copy(kT[:, sl], kp[0:D, 0:QTR])
        vp3 = vp[:, 0:8 * D].rearrange("p (c v) -> p c v", v=D)
        nc.scalar.activation(vex3[:, 8 * qq:8 * qq + 8, 0:D], vp3, AF.Copy)
        # f16 squares (DVE 2x)
        nc.vector.tensor_mul(sqq8[:, sl], qT[:, sl], qT[:, sl])
        nc.vector.tensor_mul(sqk[:, sl], kT[:, sl], kT[:, sl])

    # ---------------- inverse norms ----------------
    # selector matmuls (K=8) pack per-block column sums into partitions
    ssq_k = ps_o.tile([D, QB], F32, tag="pso", name="ssq_k")
    ssq_q = ps_s.tile([KC, GMAX * QB], F32, tag="pss", name="ssq_q")
    for ssq, sq in ((ssq_k[:], sqk[:]), (ssq_q[0:D, 0:QB], sqq8[:])):
        for j in range(NQB):
            sel_j = wpack[0:D, SEL0 + D * j:SEL0 + D * (j + 1)]
            nc.tensor.matmul(ssq, sel_j, sq[:, ts(j, QB)],
                             start=(j == 0), stop=(j == NQB - 1))
    # inv = 1/sqrt(ssq): ACT sqrt -> DVE fast reciprocal (fp32) -> f16
    # (Rsqrt activation is blocked in bass for accuracy); scratches reuse
    # prologue-dead f32 tiles.
    nc.scalar.sqrt(invr[0:D, :], ssq_k[:])
    nc.scalar.sqrt(oTe[0:D, 0:QB], ssq_q[0:D, 0:QB])
    # exp table load: input aliases the sqrt output so the scheduler
    # cannot hoist it before the sqrts (which would thrash table sets)
    nc.scalar.activation(warm[:], oTe[0:1, 0:1], AF.Exp)
    nc.vector.reciprocal_approx_fast(repE[:, 0:QB], invr[0:D, :])
    nc.vector.tensor_copy(inv_k[:], repE[:, 0:QB])
    nc.vector.reciprocal_approx_fast(repE[:, QB:2 * QB], oTe[0:D, 0:QB])
    nc.vector.tensor_copy(inv_q[:], repE[:, QB:2 * QB])

    # prj ones row (for the be/8 bias fold) copied from xTe's host ones
    # row — engine memsets cannot start at partition 8
    nc.sync.dma_start(prj[D:D + 1, :], xTe[C:C + 1, :])

    # replicate inverse norms across partitions. Critical blocks (k0,
    # q0, k1 gate the first exp windows) go through PE row-broadcast
    # matmuls — an all-engine chain with no DMA completion latency. The
    # remaining blocks ride a DRAM bounce (stride-0 broadcast read) into
    # rep tiles + two batched f16 2x normalize muls, entirely off the
    # PE/ACT critical path.
    def nmul(side_inv, src, dst, j, nm):
        rb = ps_o.tile([D, QB], F32, tag="pso", name=nm)
        selr_j = wpack[0:D, SELR0 + D * j:SELR0 + D * (j + 1)]
        nc.tensor.matmul(rb[:], selr_j, side_inv[:], start=True, stop=True)
        nc.vector.tensor_mul(dst[0:D, ts(j, QB)], src[:, ts(j, QB)], rb[:])

    nmul(inv_k, kT, kTn, 0, "rbk0")
    nmul(inv_q, qT, qTn, 0, "rbq0")
    nmul(inv_k, kT, kTn, 1, "rbk1")

    dscr_k, _ = tc.tile([D, QB], F16, space="DRAM", name="dscr_k")
    dscr_q, _ = tc.tile([D, QB], F16, space="DRAM", name="dscr_q")
    nc.sync.dma_start(dscr_k[:], inv_k[:])
    nc.sync.dma_start(dscr_q[:], inv_q[:])
    nc.sync.dma_start(
        rep_k[:, 2 * QB:],
        dscr_k[2:NQB, :].unsqueeze(0).to_broadcast((D, NQB - 2, QB)))
    nc.sync.dma_start(
        rep_q[:, QB:],
        dscr_q[1:NQB, :].unsqueeze(0).to_broadcast((D, NQB - 1, QB)))
    nc.vector.tensor_mul(kTn[0:D, 2 * QB:], kT[:, 2 * QB:],
                         rep_k[:, 2 * QB:])
    nc.vector.tensor_mul(qTn[0:D, QB:], qT[:, QB:], rep_q[:, QB:])

    # ---------------- main attention loop + dripped epilogue ------------
    dscr_e, _ = tc.tile([NQB, QB], F32, space="DRAM", name="dscr_e")
    seq = []
    for j in range(NQB):
        c = 0
        for g in GROUPS:
            seq.append((j, c, g))
            c += g
    pos = [None] * NQB
    pss = [None] * len(seq)

    def mm1(i):
        j, c, g = seq[i]
        ps = ps_s.tile([KC, GMAX * QB], F32, tag="pss", name=f"pss{i}")
        pss[i] = ps
        for u in range(g):
            nc.tensor.matmul(ps[:, ts(u, QB)], kTn[:, ts(c + u, KC)],
                             qTn[:, ts(j, QB)], start=True, stop=True)

    def epi_stages(j):
        """Per-block epilogue: 1/den, broadcast, normalize, project,
        store — dripped one stage per subsequent loop entry."""
        projps = [None]

        def s_den(j=j):
            # engine ops need quadrant-aligned partition bases, so the
            # denom row (partition 8) moves to partition 0 by DMA first
            nc.sync.dma_start(deno[:], oTe[D:D + 1, ts(j, QB)])

        def s_recip(j=j):
            nc.vector.reciprocal_approx_fast(rcpo[:], deno[:])

        def s_wr(j=j):
            nc.sync.dma_start(dscr_e[j:j + 1, :], rcpo[:])

        def s_bcast(j=j):
            nc.sync.dma_start(repE[:, ts(j, QB)],
                              dscr_e[j:j + 1, :].to_broadcast((D, QB)))

        def s_mul(j=j):
            nc.vector.tensor_mul(prj[0:D, ts(j, QB)], oTe[0:D, ts(j, QB)],
                                 repE[:, ts(j, QB)])

        def s_proj(j=j):
            ps = ps_o.tile([C, QB], F32, tag="pso", name=f"proj{j}")
            nc.tensor.matmul(ps[:], webe, prj[:, ts(j, QB)], start=True,
                             stop=True)
            projps[0] = ps

        def s_copy(j=j):
            nc.vector.tensor_copy(resT[:, ts(j, QB)], projps[0][:])

        def s_out(j=j):
            nc.sync.dma_start(out_d[:, ts(j, QB)], resT[:, ts(j, QB)])

        def s_den7(j=j):
            # last block: PSUM is free, broadcast the denom row with one
            # fp32 matmul (oneden row-8 selector) instead of 3 chained
            # DMAs with ~1.3us completion latency each
            ps = ps_s.tile([D, QB], F32, tag="pss", name="den7")
            nc.tensor.matmul(ps[:], oneden_t[:], oTe[:, ts(j, QB)],
                             start=True, stop=True)
            projps[0] = ps

        def s_recip7(j=j):
            nc.vector.reciprocal_approx_fast(repE[:, ts(j, QB)],
                                             projps[0][:])

        if j == NQB - 1:
            return [s_den7, s_recip7, s_mul, s_proj, s_copy, s_out]
        return [s_den, s_recip, s_wr, s_bcast, s_mul, s_proj, s_copy, s_out]

    pend = []

    mm1(0)
    for i, (j, c, g) in enumerate(seq):
        if pos[j] is None:
            pos[j] = ps_o.tile([VW, QB], F32, tag="pso", name=f"po{j}")
        if i + 1 < len(seq):
            mm1(i + 1)
        ps = pss[i]
        es = expp.tile([KC, GMAX * QB], F16, tag="es")
        nc.scalar.activation(es[:, 0:g * QB], ps[:, 0:g * QB], AF.Exp)
        for u in range(g):
            cc = c + u
            nc.tensor.matmul(pos[j][:], vext[:, cc * VW:(cc + 1) * VW],
                             es[:, ts(u, QB)],
                             start=(cc == 0), stop=(cc == NKC - 1))
        pss[i] = None
        if c + g == NKC:
            nc.vector.tensor_copy(oTe[:, ts(j, QB)], pos[j][:])
            pend.append(epi_stages(j))
        elif pend:
            pend[0].pop(0)()
            if not pend[0]:
                pend.pop(0)
    while pend:
        pend[0].pop(0)()
        if not pend[0]:
            pend.pop(0)


def _build():
    global _BUILT
    if _BUILT is not None:
        return _BUILT
    nc = bacc.Bacc("TRN2", target_bir_lowering=False, debug=False, num_devices=H)
    xTe_d = nc.dram_tensor("xTe", [KC, HW], F16, kind="ExternalInput").ap()
    yTe_d = nc.dram_tensor("yTe", [KC, HW], F16, kind="ExternalInput").ap()
    wpack_d = nc.dram_tensor("wpack", [KC, WPC], F16, kind="ExternalInput").ap()
    oneden_d = nc.dram_tensor("oneden", [VW, D], F32,
                              kind="ExternalInput").ap()
    out_d = nc.dram_tensor("resT", [C, HW], F32, kind="ExternalOutput").ap()
    with tile.TileContext(nc) as tc, ExitStack() as ctx:
        _body(ctx, tc, (xTe_d, yTe_d, wpack_d, oneden_d, out_d[:]))
    nc.compile()
    _BUILT = nc
    return nc


def make_in_maps(x, y, Wq, bq, Wkv, bkv, We, be):
    x, y, Wq, bq, Wkv, bkv, We, be = (
        np.asarray(a, np.float32) for a in (x, y, Wq, bq, Wkv, bkv, We, be))
    ones = np.ones((1, HW), np.float32)
    zrows = np.zeros((KC - C - 1, HW), np.float32)
    xTe = np.vstack([x[0].T, ones, zrows]).astype(np.float16)
    yTe = np.vstack([y[0].T, ones, zrows]).astype(np.float16)
    sel = np.zeros((KC, C), np.float32)
    selr = np.zeros((KC, C), np.float32)
    for j in range(NQB):
        sel[0:D, D * j + j] = 1.0          # col j of group j: ones rows 0-7
        selr[j, D * j:D * (j + 1)] = 1.0   # row j of group j: all ones
    oneden = np.zeros((VW, D), np.float32)
    oneden[D, :] = 1.0
    in_maps = []
    for h in range(H):
        sl = slice(h * D, (h + 1) * D)
        slv = slice(C + h * D, C + (h + 1) * D)
        zc = np.zeros((KC - C - 1, D), np.float32)
        wqe = np.vstack([Wq[:, sl], bq[None, sl], zc])
        wke = np.vstack([Wkv[:, sl], bkv[None, sl], zc])
        wve = np.vstack([Wkv[:, slv], bkv[None, slv], zc])
        webe = np.vstack([We[sl, :], be[None, :] / H,
                          np.zeros((KC - VW, C), np.float32)])
        wpack = np.concatenate([wqe, wke, wve, webe, sel, selr], axis=1)
        in_maps.append({
            "xTe": xTe,
            "yTe": yTe,
            "wpack": np.ascontiguousarray(wpack.astype(np.float16)),
            "oneden": oneden,
        })
    return in_maps


def kernel(x, y, Wq, bq, Wkv, bkv, We, be):
    global LAST_RESULTS
    nc = _build()
    in_maps = make_in_maps(x, y, Wq, bq, Wkv, bkv, We, be)
    res = run_bass_kernel_spmd(nc, in_maps, core_ids=list(range(H)), trace=TRACE)
    LAST_RESULTS = res
    acc = np.zeros((C, HW), np.float64)
    for r in res.results:
        acc += r["resT"]
    return np.ascontiguousarray(acc.T[None]).astype(np.float32)
